# revision 1
# baseline (speedup 1.0000x reference)
"""Multi-head causal attention (B=4, S=2048, E=1024, H=16, D=64) on 8 TRN2 cores.

Sharding: core c = (batch b = c//2, head-group g = c%2 of 8 heads).
Each core computes Q/K/V projections for its (batch, 8 heads), causal
attention (full score rows per q-tile, no online softmax), and a partial
output projection  ctx[:, g*512:(g+1)*512] @ Wo[g*512:(g+1)*512, :].
Host sums the two partials per batch and adds the bias.

Schedule: the PE instruction stream interleaves, at matmul-chain granularity,
projection chains of s-quarter sq+1 (and output-projection chains during the
last wave) between the attention k-groups of wave sq.  The attention groups
are gated by the scalar engine's exp throughput, so the woven-in projection
chains fill the PE bubbles.

Device layouts (per core):
  xt   [1024, 2048]  = X[b].T                      (e on partitions)
  kt   [128, 4, 2048]: pair p, partitions (h%2)*64+d = head-dim, free = seq
  qt   rotating [128, 512] tiles per (pair, quarter)
  v    [128, 16, 8, 65]: s-chunk tiles; per head 64 V columns + ones column
  scoresT tiles [k=128, q=512] so that exp(scores) is directly the AV lhsT
  ctxT [128, 4, 2048]: feeds the output projection as lhsT
All matmuls run as float32r (full PE rate at N>=512, ~fp32 accuracy).
Causal masking: gpsimd.affine_select zeroes the strict upper triangle of the
exp tiles on the diagonal k-groups.
"""

import os
from contextlib import ExitStack

import numpy as np

import concourse.bass as bass
from concourse import bacc
import concourse.mybir as mybir
import concourse.tile as tile
from concourse.bass_utils import run_bass_kernel_spmd

F32 = mybir.dt.float32
FR = mybir.dt.float32r

B, S, E = 4, 2048, 1024
H, D = 16, 64
NHC = 8          # heads per core
NP = 4           # head pairs per core
HDC = NHC * D    # 512 per-core head dims
AF = mybir.ActivationFunctionType

_NC = None
_LAST_RESULTS = None


def _emit(tc, stack):
    nc = tc.nc
    xt = nc.dram_tensor("xt", [E, S], FR, kind="ExternalInput").ap()
    wq = nc.dram_tensor("wq", [E, HDC], FR, kind="ExternalInput").ap()
    wk = nc.dram_tensor("wk", [E, HDC], FR, kind="ExternalInput").ap()
    wv = nc.dram_tensor("wv", [E, HDC], FR, kind="ExternalInput").ap()
    wo = nc.dram_tensor("wo", [HDC, E], FR, kind="ExternalInput").ap()
    vones = nc.dram_tensor("vones", [16, NHC], FR, kind="ExternalInput").ap()
    out = nc.dram_tensor("out", [S, E], F32, kind="ExternalOutput").ap()
    # DRAM scratch for broadcasting softmax denominators across partitions
    zscratch = nc.dram_tensor("zscratch", [NP * 4 * 2, 512], F32, kind="Internal").ap()

    persist = stack.enter_context(tc.tile_pool(name="persist", bufs=1))
    kt_sb = persist.tile([128, NP, S], FR, tag="kt")
    v_sb = persist.tile([128, 16, NHC, 65], FR, tag="v")
    ctx_sb = persist.tile([128, NP, S], FR, tag="ctx")

    # ones column for the softmax-denominator trick (memset can't write f32r)
    nc.sync.dma_start(
        out=v_sb[:, :, :, 64:65],
        in_=vones.unsqueeze(2).partition_broadcast(128),
    )

    projps = stack.enter_context(tc.tile_pool(name="projps", bufs=2, space="PSUM"))
    inner = stack.enter_context(ExitStack())
    xtpool = inner.enter_context(tc.tile_pool(name="xtpool", bufs=8))
    qtpool = inner.enter_context(tc.tile_pool(name="qtpool", bufs=8))
    expt_pool = inner.enter_context(tc.tile_pool(name="expt", bufs=5))
    recip_pool = inner.enter_context(tc.tile_pool(name="recip", bufs=2))
    scoresps = inner.enter_context(tc.tile_pool(name="scoresps", bufs=2, space="PSUM"))
    ctxps = inner.enter_context(tc.tile_pool(name="ctxps", bufs=2, space="PSUM"))
    wstack = ExitStack()
    wpool = wstack.enter_context(tc.tile_pool(name="wpool", bufs=1))

    wq_sb = wpool.tile([128, 8, HDC], FR, tag="wq")
    wk_sb = wpool.tile([128, 8, HDC], FR, tag="wk")
    wv_sb = wpool.tile([128, 8, HDC], FR, tag="wv")
    def _load_wq_and_xt0(xts):
        # weights on the HWDGE queues, xt0 on the SWDGE queues: the startup
        # is DMA-bandwidth-bound, so use both engine groups in parallel
        for k in range(8):
            for h0, h1 in ((0, 256), (256, 512)):
                nc.sync.dma_start(
                    out=wq_sb[:, k, h0:h1],
                    in_=wq[k * 128 : (k + 1) * 128, h0:h1],
                )
            nc.gpsimd.dma_start(
                out=xts[k], in_=xt[k * 128 : (k + 1) * 128, 0:512]
            )
    def _load_wkv():
        for k in range(8):
            nc.sync.dma_start(
                out=wk_sb[:, k, :], in_=wk[k * 128 : (k + 1) * 128, :]
            )
        for k in range(8):
            nc.sync.dma_start(
                out=wv_sb[:, k, :], in_=wv[k * 128 : (k + 1) * 128, :]
            )

    qts = {}  # (sq, pair) -> qt tile

    def load_xt_quarter(sq):
        s0 = sq * 512
        xts = []
        for k in range(8):
            xtt = xtpool.tile([128, 512], FR, tag="xt", name=f"xt{sq}_{k}")
            nc.sync.dma_start(
                out=xtt, in_=xt[k * 128 : (k + 1) * 128, s0 : s0 + 512]
            )
            xts.append(xtt)
        return xts

    def proj_chains(sq, xts):
        """Yield 12 chain-emitters for s-quarter sq: 4 V, 4 QT, 4 KT."""
        s0 = sq * 512

        def v_chain(sc2):
            def emit():
                sc = 4 * sq + sc2
                ps = projps.tile([128, 512], F32, tag="pp", name=f"psv{sq}_{sc2}")
                for k in range(8):
                    nc.tensor.matmul(
                        out=ps,
                        lhsT=xts[k][:, sc2 * 128 : (sc2 + 1) * 128],
                        rhs=wv_sb[:, k, :],
                        start=(k == 0),
                        stop=(k == 7),
                    )
                nc.vector.tensor_copy(
                    out=v_sb[:, sc, :, 0:64],
                    in_=ps.rearrange("p (h d) -> p h d", d=64),
                )
            return emit

        def q_chain(m):
            def emit():
                ps = projps.tile([128, 512], F32, tag="pp", name=f"psq{sq}_{m}")
                for k in range(8):
                    nc.tensor.matmul(
                        out=ps,
                        lhsT=wq_sb[:, k, m * 128 : (m + 1) * 128],
                        rhs=xts[k],
                        start=(k == 0),
                        stop=(k == 7),
                    )
                qtt = qtpool.tile([128, 512], FR, tag="qt", name=f"qt{sq}_{m}")
                nc.vector.tensor_copy(out=qtt, in_=ps)
                qts[(sq, m)] = qtt
            return emit

        def k_chain(m):
            def emit():
                ps = projps.tile([128, 512], F32, tag="pp", name=f"psk{sq}_{m}")
                for k in range(8):
                    nc.tensor.matmul(
                        out=ps,
                        lhsT=wk_sb[:, k, m * 128 : (m + 1) * 128],
                        rhs=xts[k],
                        start=(k == 0),
                        stop=(k == 7),
                    )
                nc.vector.tensor_copy(out=kt_sb[:, m, s0 : s0 + 512], in_=ps)
            return emit

        # Q first so wave sq-1's tail can overlap; K/V next
        return (
            [q_chain(m) for m in range(NP)]
            + [k_chain(m) for m in range(NP)]
            + [v_chain(c) for c in range(4)]
        )

    wo_sb = None
    stg_pool = None

    def oproj_chain(sc, n):
        def emit():
            ps = projps.tile([128, 512], F32, tag="pp", name=f"pso{sc}_{n}")
            for kp in range(4):
                nc.tensor.matmul(
                    out=ps,
                    lhsT=ctx_sb[:, kp, sc * 128 : (sc + 1) * 128],
                    rhs=wo_sb[:, kp, n * 512 : (n + 1) * 512],
                    start=(kp == 0),
                    stop=(kp == 3),
                )
            st = stg_pool.tile([128, 512], F32, tag="stg", name=f"st{sc}_{n}")
            nc.vector.tensor_copy(out=st, in_=ps)
            nc.sync.dma_start(
                out=out[sc * 128 : (sc + 1) * 128, n * 512 : (n + 1) * 512],
                in_=st,
            )
        return emit

    def attention_wave(t, fillers):
        """Emit wave t's attention groups, weaving `fillers` chain-emitters
        between k-groups."""
        q0 = t * 512
        ngroups = 2 * (t + 1)  # k-groups of 2 k-tiles
        total_groups = NP * ngroups
        gi = 0
        nf = len(fillers)
        fi = 0
        def _emit_av(exp_t, g, p, cps):
            for hh in range(2):
                for kk in range(2):
                    j = 2 * g + kk
                    nc.tensor.matmul(
                        out=cps[hh],
                        lhsT=v_sb[:, j, 2 * p + hh, :],
                        rhs=exp_t[hh][:, kk * 512 : (kk + 1) * 512],
                        start=(g == 0 and kk == 0),
                        stop=(g == ngroups - 1 and kk == 1),
                    )

        def _normalize(p, cps):
            # stage the raw ctx to SBUF immediately so the PSUM accumulator
            # bank frees before the denominator's DRAM round-trip completes
            for hh in range(2):
                h64 = hh * 64
                rc = recip_pool.tile([1, 512], F32, tag="recip", name=f"rc{p}{t}{hh}", bufs=1)
                nc.vector.reciprocal(out=rc, in_=cps[hh][64:65, :])
                cstg = recip_pool.tile(
                    [64, 512], F32, tag="cstg", name=f"cs{p}{t}{hh}"
                )
                nc.vector.tensor_copy(out=cstg, in_=cps[hh][0:64, :])
                u = (p * 4 + t) * 2 + hh
                nc.sync.dma_start(out=zscratch[u : u + 1, :], in_=rc)
                rcb = recip_pool.tile(
                    [64, 512], F32, tag="recipb", name=f"rcb{p}{t}{hh}"
                )
                nc.sync.dma_start(
                    out=rcb, in_=zscratch[u : u + 1, :].partition_broadcast(64)
                )
                nc.vector.tensor_mul(
                    out=ctx_sb[h64 : h64 + 64, p, q0 : q0 + 512],
                    in0=cstg,
                    in1=rcb,
                )

        pending = None  # (exp_t, g, p, ctx_ps)
        ctx_ps = None
        for p in range(NP):
            ctx_ps = [
                ctxps.tile([65, 512], F32, tag="ctxps", name=f"ctxps{p}_{t}_{i}")
                for i in range(2)
            ]
            for g in range(ngroups):
                # weave fillers evenly across the wave
                while fi < nf and fi * total_groups <= gi * nf:
                    fillers[fi]()
                    fi += 1
                gi += 1
                sc_ps = [
                    scoresps.tile(
                        [128, 1024], F32, tag="scores", name=f"sc{p}_{t}_{g}_{i}"
                    )
                    for i in range(2)
                ]
                for kk in range(2):
                    j = 2 * g + kk
                    for hh in range(2):
                        h64 = hh * 64
                        nc.tensor.matmul(
                            out=sc_ps[hh][:, kk * 512 : (kk + 1) * 512],
                            lhsT=kt_sb[h64 : h64 + 64, p, j * 128 : (j + 1) * 128],
                            rhs=qts[(t, p)][h64 : h64 + 64, :],
                            start=True,
                            stop=True,
                        )
                exp_t = [None, None]
                for hh in range(2):
                    et = expt_pool.tile(
                        [128, 1024], FR, tag="expt", name=f"et{p}_{t}_{g}_{hh}"
                    )
                    nc.scalar.activation(
                        out=et, in_=sc_ps[hh], func=AF.Exp, scale=0.125
                    )
                    exp_t[hh] = et
                if g >= 2 * t:  # diagonal band -> zero causal upper triangle
                    # valid iff qf - kp - 128*(2*(g-2t) + kk) >= 0
                    for hh in range(2):
                        nc.gpsimd.affine_select(
                            out=exp_t[hh],
                            in_=exp_t[hh],
                            compare_op=mybir.AluOpType.is_ge,
                            fill=0.0,
                            base=-256 * (g - 2 * t),
                            pattern=[[-128, 2], [1, 512]],
                            channel_multiplier=-1,
                        )
                # software pipeline: issue the PREVIOUS group's AV matmuls so
                # the PE never sits on this group's exp latency; when that
                # was a pair's last group, its normalization follows
                if pending is not None:
                    _emit_av(*pending)
                    if pending[1] == ngroups - 1:
                        _normalize(pending[2], pending[3])
                pending = (exp_t, g, p, ctx_ps)
        if pending is not None:
            _emit_av(*pending)
            _normalize(pending[2], pending[3])
            pending = None
        # leftover fillers
        while fi < nf:
            fillers[fi]()
            fi += 1

    # quarter 0 projections run un-woven (nothing to overlap with yet)
    xts0 = [
        xtpool.tile([128, 512], FR, tag="xt", name=f"xt0_{k}") for k in range(8)
    ]
    _load_wq_and_xt0(xts0)
    xts1 = load_xt_quarter(1)  # queued before wk/wv: needed by wave 0's fillers
    _load_wkv()
    for emit in proj_chains(0, xts0):
        emit()
    # waves 0..2 weave the next quarter's projection chains
    xts_next = xts1
    for t in range(3):
        chains = proj_chains(t + 1, xts_next)
        if t + 2 <= 3:
            pass
        attention_wave(t, chains)
        if t + 2 <= 3:
            xts_next = load_xt_quarter(t + 2)
    # weights for q/k/v no longer needed; free for the output projection
    wstack.close()
    ostack = stack.enter_context(ExitStack())
    opool = ostack.enter_context(tc.tile_pool(name="opool", bufs=1))
    stg_pool = ostack.enter_context(tc.tile_pool(name="stg", bufs=3))
    wo_sb = opool.tile([128, 4, E], FR, tag="wo")
    nc.sync.dma_start(out=wo_sb, in_=wo.rearrange("(k p) n -> p k n", p=128))
    # wave 3 weaves output-projection chains for s-chunks 0..11 (q < 1536,
    # whose ctxT rows are complete after waves 0..2)
    fillers3 = [oproj_chain(sc, n) for sc in range(12) for n in range(2)]
    # hold back twelve independent chains to cover the final normalize latency
    held = fillers3[-12:]
    attention_wave(3, fillers3[:-12])
    for emit in held:
        emit()
    # tail: s-chunks 12..15 need wave 3's ctxT
    for sc in range(12, 16):
        for n in range(2):
            oproj_chain(sc, n)()


def _build():
    global _NC
    if _NC is None:
        nc = bacc.Bacc("TRN2", target_bir_lowering=False, debug=False)
        with tile.TileContext(nc) as tc, ExitStack() as stack:
            _emit(tc, stack)
        if not nc.is_finalized():
            nc.finalize()
        _NC = nc
    return _NC


def kernel(X, Wq, Wk, Wv, Wo, bo):
    global _LAST_RESULTS
    X = np.ascontiguousarray(np.asarray(X, dtype=np.float32))
    Wq = np.asarray(Wq, dtype=np.float32)
    Wk = np.asarray(Wk, dtype=np.float32)
    Wv = np.asarray(Wv, dtype=np.float32)
    Wo = np.asarray(Wo, dtype=np.float32)
    bo = np.asarray(bo, dtype=np.float32)

    nc = _build()
    XT = np.ascontiguousarray(X.transpose(0, 2, 1))  # [B, E, S]
    in_maps = []
    for c in range(8):
        b, g = c // 2, c % 2
        cs = slice(g * HDC, (g + 1) * HDC)
        in_maps.append(
            {
                "xt": XT[b],
                "wq": np.ascontiguousarray(Wq[:, cs]),
                "wk": np.ascontiguousarray(Wk[:, cs]),
                "wv": np.ascontiguousarray(Wv[:, cs]),
                "wo": np.ascontiguousarray(Wo[cs, :]),
                "vones": np.ones((16, NHC), dtype=np.float32),
            }
        )
    trace = bool(int(os.environ.get("KTRACE", "0")))
    res = run_bass_kernel_spmd(
        nc, in_maps, core_ids=list(range(8)), trace=trace
    )
    _LAST_RESULTS = res
    out = np.empty((B, S, E), dtype=np.float32)
    for b in range(B):
        out[b] = res.results[2 * b]["out"] + res.results[2 * b + 1]["out"] + bo
    return out



# revision 6
# speedup vs baseline: 1.8979x; 1.8979x over previous
"""Multi-head causal attention (B=4, S=2048, E=1024, H=16, D=64) on 8 TRN2 cores.

Sharding: core c = (batch b = c//2, head-group g = c%2 of 8 heads).
Each core computes Q/K/V projections for its (batch, 8 heads), causal
attention (full score rows per q-tile, no online softmax), and a partial
output projection  ctx[:, g*512:(g+1)*512] @ Wo[g*512:(g+1)*512, :].
Host sums the two partials per batch and adds the bias.

Schedule: the PE instruction stream interleaves, at matmul-chain granularity,
projection chains of s-quarter sq+1 (and output-projection chains during the
last wave) between the attention k-groups of wave sq.  The attention groups
are gated by the scalar engine's exp throughput, so the woven-in projection
chains fill the PE bubbles.

Device layouts (per core):
  xt   [1024, 2048]  = X[b].T                      (e on partitions)
  kt   [128, 4, 2048]: pair p, partitions (h%2)*64+d = head-dim, free = seq
  qt   rotating [128, 512] tiles per (pair, quarter)
  v    [128, 16, 8, 65]: s-chunk tiles; per head 64 V columns + ones column
  scoresT tiles [k=128, q=512] so that exp(scores) is directly the AV lhsT
  ctxT [128, 4, 2048]: feeds the output projection as lhsT
All matmuls run as float32r (full PE rate at N>=512, ~fp32 accuracy).
Causal masking: gpsimd.affine_select zeroes the strict upper triangle of the
exp tiles on the diagonal k-groups.
"""

import os
from contextlib import ExitStack

import numpy as np

import concourse.bass as bass
from concourse import bacc
import concourse.mybir as mybir
import concourse.tile as tile
from concourse.bass_utils import run_bass_kernel_spmd

F32 = mybir.dt.float32
FR = mybir.dt.float16  # wire/SBUF dtype: fp16 halves tunnel bytes, 2x PE rate

B, S, E = 4, 2048, 1024
H, D = 16, 64
NHC = 8          # heads per core
NP = 4           # head pairs per core
HDC = NHC * D    # 512 per-core head dims
AF = mybir.ActivationFunctionType

_NC = None
_LAST_RESULTS = None


def _emit(tc, stack):
    nc = tc.nc
    xt = nc.dram_tensor("xt", [E, S], FR, kind="ExternalInput").ap()
    wq = nc.dram_tensor("wq", [E, HDC], FR, kind="ExternalInput").ap()
    wk = nc.dram_tensor("wk", [E, HDC], FR, kind="ExternalInput").ap()
    wv = nc.dram_tensor("wv", [E, HDC], FR, kind="ExternalInput").ap()
    wo = nc.dram_tensor("wo", [HDC, E], FR, kind="ExternalInput").ap()
    vones = nc.dram_tensor("vones", [16, NHC], FR, kind="ExternalInput").ap()
    out = nc.dram_tensor("out", [S, E], FR, kind="ExternalOutput").ap()
    # DRAM scratch for broadcasting softmax denominators across partitions
    zscratch = nc.dram_tensor("zscratch", [NP * 4 * 2, 512], F32, kind="Internal").ap()

    persist = stack.enter_context(tc.tile_pool(name="persist", bufs=1))
    kt_sb = persist.tile([128, NP, S], FR, tag="kt")
    v_sb = persist.tile([128, 16, NHC, 65], FR, tag="v")
    ctx_sb = persist.tile([128, NP, S], FR, tag="ctx")

    # ones column for the softmax-denominator trick (memset can't write f32r)
    nc.sync.dma_start(
        out=v_sb[:, :, :, 64:65],
        in_=vones.unsqueeze(2).partition_broadcast(128),
    )

    projps = stack.enter_context(tc.tile_pool(name="projps", bufs=2, space="PSUM"))
    inner = stack.enter_context(ExitStack())
    xtpool = inner.enter_context(tc.tile_pool(name="xtpool", bufs=8))
    qtpool = inner.enter_context(tc.tile_pool(name="qtpool", bufs=8))
    expt_pool = inner.enter_context(tc.tile_pool(name="expt", bufs=5))
    recip_pool = inner.enter_context(tc.tile_pool(name="recip", bufs=2))
    scoresps = inner.enter_context(tc.tile_pool(name="scoresps", bufs=2, space="PSUM"))
    ctxps = inner.enter_context(tc.tile_pool(name="ctxps", bufs=2, space="PSUM"))
    wstack = ExitStack()
    wpool = wstack.enter_context(tc.tile_pool(name="wpool", bufs=1))

    wq_sb = wpool.tile([128, 8, HDC], FR, tag="wq")
    wk_sb = wpool.tile([128, 8, HDC], FR, tag="wk")
    wv_sb = wpool.tile([128, 8, HDC], FR, tag="wv")
    def _load_wq_and_xt0(xts):
        # weights on the HWDGE queues, xt0 on the SWDGE queues: the startup
        # is DMA-bandwidth-bound, so use both engine groups in parallel
        for k in range(8):
            for h0, h1 in ((0, 256), (256, 512)):
                nc.sync.dma_start(
                    out=wq_sb[:, k, h0:h1],
                    in_=wq[k * 128 : (k + 1) * 128, h0:h1],
                )
            nc.gpsimd.dma_start(
                out=xts[k], in_=xt[k * 128 : (k + 1) * 128, 0:512]
            )
    def _load_wkv():
        for k in range(8):
            nc.sync.dma_start(
                out=wk_sb[:, k, :], in_=wk[k * 128 : (k + 1) * 128, :]
            )
        for k in range(8):
            nc.sync.dma_start(
                out=wv_sb[:, k, :], in_=wv[k * 128 : (k + 1) * 128, :]
            )

    qts = {}  # (sq, pair) -> qt tile

    def load_xt_quarter(sq):
        s0 = sq * 512
        xts = []
        for k in range(8):
            xtt = xtpool.tile([128, 512], FR, tag="xt", name=f"xt{sq}_{k}")
            nc.sync.dma_start(
                out=xtt, in_=xt[k * 128 : (k + 1) * 128, s0 : s0 + 512]
            )
            xts.append(xtt)
        return xts

    def proj_chains(sq, xts):
        """Yield 12 chain-emitters for s-quarter sq: 4 V, 4 QT, 4 KT."""
        s0 = sq * 512

        def v_chain(sc2):
            def emit():
                sc = 4 * sq + sc2
                ps = projps.tile([128, 512], F32, tag="pp", name=f"psv{sq}_{sc2}")
                for k in range(8):
                    nc.tensor.matmul(
                        out=ps,
                        lhsT=xts[k][:, sc2 * 128 : (sc2 + 1) * 128],
                        rhs=wv_sb[:, k, :],
                        start=(k == 0),
                        stop=(k == 7),
                    )
                nc.vector.tensor_copy(
                    out=v_sb[:, sc, :, 0:64],
                    in_=ps.rearrange("p (h d) -> p h d", d=64),
                )
            return emit

        def q_chain(m):
            def emit():
                ps = projps.tile([128, 512], F32, tag="pp", name=f"psq{sq}_{m}")
                for k in range(8):
                    nc.tensor.matmul(
                        out=ps,
                        lhsT=wq_sb[:, k, m * 128 : (m + 1) * 128],
                        rhs=xts[k],
                        start=(k == 0),
                        stop=(k == 7),
                    )
                qtt = qtpool.tile([128, 512], FR, tag="qt", name=f"qt{sq}_{m}")
                nc.vector.tensor_copy(out=qtt, in_=ps)
                qts[(sq, m)] = qtt
            return emit

        def k_chain(m):
            def emit():
                ps = projps.tile([128, 512], F32, tag="pp", name=f"psk{sq}_{m}")
                for k in range(8):
                    nc.tensor.matmul(
                        out=ps,
                        lhsT=wk_sb[:, k, m * 128 : (m + 1) * 128],
                        rhs=xts[k],
                        start=(k == 0),
                        stop=(k == 7),
                    )
                nc.vector.tensor_copy(out=kt_sb[:, m, s0 : s0 + 512], in_=ps)
            return emit

        # Q first so wave sq-1's tail can overlap; K/V next
        return (
            [q_chain(m) for m in range(NP)]
            + [k_chain(m) for m in range(NP)]
            + [v_chain(c) for c in range(4)]
        )

    wo_sb = None
    stg_pool = None

    def oproj_chain(sc, n):
        def emit():
            ps = projps.tile([128, 512], F32, tag="pp", name=f"pso{sc}_{n}")
            for kp in range(4):
                nc.tensor.matmul(
                    out=ps,
                    lhsT=ctx_sb[:, kp, sc * 128 : (sc + 1) * 128],
                    rhs=wo_sb[:, kp, n * 512 : (n + 1) * 512],
                    start=(kp == 0),
                    stop=(kp == 3),
                )
            st = stg_pool.tile([128, 512], FR, tag="stg", name=f"st{sc}_{n}")
            nc.vector.tensor_copy(out=st, in_=ps)
            nc.sync.dma_start(
                out=out[sc * 128 : (sc + 1) * 128, n * 512 : (n + 1) * 512],
                in_=st,
            )
        return emit

    def attention_wave(t, fillers):
        """Emit wave t's attention groups, weaving `fillers` chain-emitters
        between k-groups."""
        q0 = t * 512
        ngroups = 2 * (t + 1)  # k-groups of 2 k-tiles
        total_groups = NP * ngroups
        gi = 0
        nf = len(fillers)
        fi = 0
        def _emit_av(exp_t, g, p, cps):
            for hh in range(2):
                for kk in range(2):
                    j = 2 * g + kk
                    nc.tensor.matmul(
                        out=cps[hh],
                        lhsT=v_sb[:, j, 2 * p + hh, :],
                        rhs=exp_t[hh][:, kk * 512 : (kk + 1) * 512],
                        start=(g == 0 and kk == 0),
                        stop=(g == ngroups - 1 and kk == 1),
                    )

        def _normalize(p, cps):
            # stage the raw ctx to SBUF immediately so the PSUM accumulator
            # bank frees before the denominator's DRAM round-trip completes
            for hh in range(2):
                h64 = hh * 64
                rc = recip_pool.tile([1, 512], F32, tag="recip", name=f"rc{p}{t}{hh}", bufs=1)
                nc.vector.reciprocal(out=rc, in_=cps[hh][64:65, :])
                cstg = recip_pool.tile(
                    [64, 512], F32, tag="cstg", name=f"cs{p}{t}{hh}"
                )
                nc.vector.tensor_copy(out=cstg, in_=cps[hh][0:64, :])
                u = (p * 4 + t) * 2 + hh
                nc.sync.dma_start(out=zscratch[u : u + 1, :], in_=rc)
                rcb = recip_pool.tile(
                    [64, 512], F32, tag="recipb", name=f"rcb{p}{t}{hh}"
                )
                nc.sync.dma_start(
                    out=rcb, in_=zscratch[u : u + 1, :].partition_broadcast(64)
                )
                nc.vector.tensor_mul(
                    out=ctx_sb[h64 : h64 + 64, p, q0 : q0 + 512],
                    in0=cstg,
                    in1=rcb,
                )

        pending = None  # (exp_t, g, p, ctx_ps)
        ctx_ps = None
        for p in range(NP):
            ctx_ps = [
                ctxps.tile([65, 512], F32, tag="ctxps", name=f"ctxps{p}_{t}_{i}")
                for i in range(2)
            ]
            for g in range(ngroups):
                # weave fillers evenly across the wave
                while fi < nf and fi * total_groups <= gi * nf:
                    fillers[fi]()
                    fi += 1
                gi += 1
                sc_ps = [
                    scoresps.tile(
                        [128, 1024], F32, tag="scores", name=f"sc{p}_{t}_{g}_{i}"
                    )
                    for i in range(2)
                ]
                for kk in range(2):
                    j = 2 * g + kk
                    for hh in range(2):
                        h64 = hh * 64
                        nc.tensor.matmul(
                            out=sc_ps[hh][:, kk * 512 : (kk + 1) * 512],
                            lhsT=kt_sb[h64 : h64 + 64, p, j * 128 : (j + 1) * 128],
                            rhs=qts[(t, p)][h64 : h64 + 64, :],
                            start=True,
                            stop=True,
                        )
                exp_t = [None, None]
                for hh in range(2):
                    et = expt_pool.tile(
                        [128, 1024], FR, tag="expt", name=f"et{p}_{t}_{g}_{hh}"
                    )
                    nc.scalar.activation(
                        out=et, in_=sc_ps[hh], func=AF.Exp, scale=0.125
                    )
                    exp_t[hh] = et
                if g >= 2 * t:  # diagonal band -> zero causal upper triangle
                    # valid iff qf - kp - 128*(2*(g-2t) + kk) >= 0
                    for hh in range(2):
                        nc.gpsimd.affine_select(
                            out=exp_t[hh],
                            in_=exp_t[hh],
                            compare_op=mybir.AluOpType.is_ge,
                            fill=0.0,
                            base=-256 * (g - 2 * t),
                            pattern=[[-128, 2], [1, 512]],
                            channel_multiplier=-1,
                        )
                # software pipeline: issue the PREVIOUS group's AV matmuls so
                # the PE never sits on this group's exp latency; when that
                # was a pair's last group, its normalization follows
                if pending is not None:
                    _emit_av(*pending)
                    if pending[1] == ngroups - 1:
                        _normalize(pending[2], pending[3])
                pending = (exp_t, g, p, ctx_ps)
        if pending is not None:
            _emit_av(*pending)
            _normalize(pending[2], pending[3])
            pending = None
        # leftover fillers
        while fi < nf:
            fillers[fi]()
            fi += 1

    # quarter 0 projections run un-woven (nothing to overlap with yet)
    xts0 = [
        xtpool.tile([128, 512], FR, tag="xt", name=f"xt0_{k}") for k in range(8)
    ]
    _load_wq_and_xt0(xts0)
    xts1 = load_xt_quarter(1)  # queued before wk/wv: needed by wave 0's fillers
    _load_wkv()
    for emit in proj_chains(0, xts0):
        emit()
    # waves 0..2 weave the next quarter's projection chains
    xts_next = xts1
    for t in range(3):
        chains = proj_chains(t + 1, xts_next)
        if t + 2 <= 3:
            pass
        attention_wave(t, chains)
        if t + 2 <= 3:
            xts_next = load_xt_quarter(t + 2)
    # weights for q/k/v no longer needed; free for the output projection
    wstack.close()
    ostack = stack.enter_context(ExitStack())
    opool = ostack.enter_context(tc.tile_pool(name="opool", bufs=1))
    stg_pool = ostack.enter_context(tc.tile_pool(name="stg", bufs=3))
    wo_sb = opool.tile([128, 4, E], FR, tag="wo")
    nc.sync.dma_start(out=wo_sb, in_=wo.rearrange("(k p) n -> p k n", p=128))
    # wave 3 weaves output-projection chains for s-chunks 0..11 (q < 1536,
    # whose ctxT rows are complete after waves 0..2)
    fillers3 = [oproj_chain(sc, n) for sc in range(12) for n in range(2)]
    # hold back twelve independent chains to cover the final normalize latency
    held = fillers3[-12:]
    attention_wave(3, fillers3[:-12])
    for emit in held:
        emit()
    # tail: s-chunks 12..15 need wave 3's ctxT
    for sc in range(12, 16):
        for n in range(2):
            oproj_chain(sc, n)()


def _build():
    global _NC
    if _NC is None:
        nc = bacc.Bacc("TRN2", target_bir_lowering=False, debug=False)
        with tile.TileContext(nc) as tc, ExitStack() as stack:
            _emit(tc, stack)
        if not nc.is_finalized():
            nc.finalize()
        _NC = nc
    return _NC


def kernel(X, Wq, Wk, Wv, Wo, bo):
    global _LAST_RESULTS
    X = np.ascontiguousarray(np.asarray(X, dtype=np.float32))
    Wq = np.asarray(Wq, dtype=np.float32)
    Wk = np.asarray(Wk, dtype=np.float32)
    Wv = np.asarray(Wv, dtype=np.float32)
    Wo = np.asarray(Wo, dtype=np.float32)
    bo = np.asarray(bo, dtype=np.float32)

    nc = _build()
    XT = X.transpose(0, 2, 1).astype(np.float16)  # [B, E, S], contiguous
    Wq16 = Wq.astype(np.float16)
    Wk16 = Wk.astype(np.float16)
    Wv16 = Wv.astype(np.float16)
    Wo16 = Wo.astype(np.float16)
    in_maps = []
    for c in range(8):
        b, g = c // 2, c % 2
        cs = slice(g * HDC, (g + 1) * HDC)
        in_maps.append(
            {
                "xt": XT[b],
                "wq": np.ascontiguousarray(Wq16[:, cs]),
                "wk": np.ascontiguousarray(Wk16[:, cs]),
                "wv": np.ascontiguousarray(Wv16[:, cs]),
                "wo": np.ascontiguousarray(Wo16[cs, :]),
                "vones": np.ones((16, NHC), dtype=np.float16),
            }
        )
    trace = bool(int(os.environ.get("KTRACE", "0")))
    res = run_bass_kernel_spmd(
        nc, in_maps, core_ids=list(range(8)), trace=trace
    )
    _LAST_RESULTS = res
    out = np.empty((B, S, E), dtype=np.float32)
    for b in range(B):
        out[b] = (
            res.results[2 * b]["out"].astype(np.float32)
            + res.results[2 * b + 1]["out"].astype(np.float32)
            + bo
        )
    return out



# revision 13
# speedup vs baseline: 4.1699x; 2.1971x over previous
"""Multi-head causal attention (B=4, S=2048, E=1024, H=16, D=64) on 8 TRN2 cores.

The run is host-tunnel-bound (slow PJRT link to the remote cores), so all
I/O is fp16 and carries only unique bytes: each core uploads half of its
batch's X^T and a quarter of its head-group's weights; pair/quad AllGathers
reconstruct the full operands on device, and a pair ReduceScatter sums the
output-projection partials so each core downloads a disjoint [S/2, E] tile.

Sharding: core c = (batch b = c//2, head-group g = c%2 of 8 heads).
Each core computes Q/K/V projections for its (batch, 8 heads), causal
attention (full score rows per q-tile, no online softmax), and a partial
output projection  ctx[:, g*512:(g+1)*512] @ Wo[g*512:(g+1)*512, :].
Host sums the two partials per batch and adds the bias.

Schedule: the PE instruction stream interleaves, at matmul-chain granularity,
projection chains of s-quarter sq+1 (and output-projection chains during the
last wave) between the attention k-groups of wave sq.  The attention groups
are gated by the scalar engine's exp throughput, so the woven-in projection
chains fill the PE bubbles.

Device layouts (per core):
  xt   [1024, 2048]  = X[b].T                      (e on partitions)
  kt   [128, 4, 2048]: pair p, partitions (h%2)*64+d = head-dim, free = seq
  qt   rotating [128, 512] tiles per (pair, quarter)
  v    [128, 16, 8, 65]: s-chunk tiles; per head 64 V columns + ones column
  scoresT tiles [k=128, q=512] so that exp(scores) is directly the AV lhsT
  ctxT [128, 4, 2048]: feeds the output projection as lhsT
All matmuls run as float32r (full PE rate at N>=512, ~fp32 accuracy).
Causal masking: gpsimd.affine_select zeroes the strict upper triangle of the
exp tiles on the diagonal k-groups.
"""

import os
from contextlib import ExitStack

import numpy as np

import concourse.bass as bass
from concourse import bacc
import concourse.mybir as mybir
import concourse.tile as tile
from concourse.bass_utils import run_bass_kernel_spmd

F32 = mybir.dt.float32
FR = mybir.dt.float16  # wire/SBUF dtype: fp16 halves tunnel bytes, 2x PE rate

B, S, E = 4, 2048, 1024
H, D = 16, 64
NHC = 8          # heads per core
NP = 4           # head pairs per core
HDC = NHC * D    # 512 per-core head dims
AF = mybir.ActivationFunctionType

_NC = None
_LAST_RESULTS = None


def _emit(tc, stack):
    nc = tc.nc
    # Per-core uploads carry only UNIQUE bytes; duplicates are reconstructed
    # on device over NeuronLink with replica-grouped AllGathers:
    #   xsh: half of XT[b] (pair group {2b, 2b+1} shares batch b)
    #   wsh: quarter of [Wq|Wk|Wv|Wo-slice] pack (quad group {g, g+2, g+4, g+6}
    #        shares head-group g)
    # The output partial is pair-ReduceScatter'ed on device so each core
    # downloads a disjoint [S/2, E] fp16 tile.
    xsh = nc.dram_tensor("xsh", [E // 2, S], FR, kind="ExternalInput").ap()
    wsh = nc.dram_tensor("wsh", [E, HDC], FR, kind="ExternalInput").ap()
    vones = nc.dram_tensor("vones", [16, NHC], FR, kind="ExternalInput").ap()
    out = nc.dram_tensor("out", [S // 2, E], FR, kind="ExternalOutput").ap()
    # DRAM scratch for broadcasting softmax denominators across partitions
    zscratch = nc.dram_tensor("zscratch", [NP * 4 * 2, 512], F32, kind="Internal").ap()

    # Internal DRAM for collective operands (collectives can't touch I/O tensors)
    xb = nc.dram_tensor("xb", [E // 2, S], FR, kind="Internal").ap()
    xt = nc.dram_tensor("xt_full", [E, S], FR, kind="Internal").ap()
    wb = nc.dram_tensor("wb", [E, HDC], FR, kind="Internal").ap()
    wfull = nc.dram_tensor("wfull", [4 * E, HDC], FR, kind="Internal").ap()
    pout = nc.dram_tensor("pout", [S, E], FR, kind="Internal").ap()
    rsout = nc.dram_tensor("rsout", [S // 2, E], FR, kind="Internal").ap()

    nc.gpsimd.dma_start(out=xb, in_=xsh)
    nc.gpsimd.dma_start(out=wb, in_=wsh)
    nc.gpsimd.collective_compute(
        "AllGather", mybir.AluOpType.bypass,
        replica_groups=[[0, 1], [2, 3], [4, 5], [6, 7]],
        ins=[xb], outs=[xt],
    )
    nc.gpsimd.collective_compute(
        "AllGather", mybir.AluOpType.bypass,
        replica_groups=[[0, 2, 4, 6], [1, 3, 5, 7]],
        ins=[wb], outs=[wfull],
    )
    wq = wfull[0 * E : 1 * E, :]
    wk = wfull[1 * E : 2 * E, :]
    wv = wfull[2 * E : 3 * E, :]
    # rows [3E, 4E) hold Wo[cs, :] ([HDC, E] row-major) packed as [E, HDC]
    wo = wfull[3 * E : 4 * E, :].rearrange("(a b) c -> a (b c)", b=2)

    persist = stack.enter_context(tc.tile_pool(name="persist", bufs=1))
    kt_sb = persist.tile([128, NP, S], FR, tag="kt")
    v_sb = persist.tile([128, 16, NHC, 65], FR, tag="v")
    ctx_sb = persist.tile([128, NP, S], FR, tag="ctx")

    # ones column for the softmax-denominator trick (memset can't write f32r)
    nc.sync.dma_start(
        out=v_sb[:, :, :, 64:65],
        in_=vones.unsqueeze(2).partition_broadcast(128),
    )

    projps = stack.enter_context(tc.tile_pool(name="projps", bufs=2, space="PSUM"))
    inner = stack.enter_context(ExitStack())
    xtpool = inner.enter_context(tc.tile_pool(name="xtpool", bufs=8))
    qtpool = inner.enter_context(tc.tile_pool(name="qtpool", bufs=8))
    expt_pool = inner.enter_context(tc.tile_pool(name="expt", bufs=5))
    recip_pool = inner.enter_context(tc.tile_pool(name="recip", bufs=2))
    scoresps = inner.enter_context(tc.tile_pool(name="scoresps", bufs=2, space="PSUM"))
    ctxps = inner.enter_context(tc.tile_pool(name="ctxps", bufs=2, space="PSUM"))
    wstack = ExitStack()
    wpool = wstack.enter_context(tc.tile_pool(name="wpool", bufs=1))

    wq_sb = wpool.tile([128, 8, HDC], FR, tag="wq")
    wk_sb = wpool.tile([128, 8, HDC], FR, tag="wk")
    wv_sb = wpool.tile([128, 8, HDC], FR, tag="wv")
    def _load_wq_and_xt0(xts):
        # weights on the HWDGE queues, xt0 on the SWDGE queues: the startup
        # is DMA-bandwidth-bound, so use both engine groups in parallel
        for k in range(8):
            for h0, h1 in ((0, 256), (256, 512)):
                nc.sync.dma_start(
                    out=wq_sb[:, k, h0:h1],
                    in_=wq[k * 128 : (k + 1) * 128, h0:h1],
                )
            nc.gpsimd.dma_start(
                out=xts[k], in_=xt[k * 128 : (k + 1) * 128, 0:512]
            )
    def _load_wkv():
        for k in range(8):
            nc.sync.dma_start(
                out=wk_sb[:, k, :], in_=wk[k * 128 : (k + 1) * 128, :]
            )
        for k in range(8):
            nc.sync.dma_start(
                out=wv_sb[:, k, :], in_=wv[k * 128 : (k + 1) * 128, :]
            )

    qts = {}  # (sq, pair) -> qt tile

    def load_xt_quarter(sq):
        s0 = sq * 512
        xts = []
        for k in range(8):
            xtt = xtpool.tile([128, 512], FR, tag="xt", name=f"xt{sq}_{k}")
            nc.sync.dma_start(
                out=xtt, in_=xt[k * 128 : (k + 1) * 128, s0 : s0 + 512]
            )
            xts.append(xtt)
        return xts

    def proj_chains(sq, xts):
        """Yield 12 chain-emitters for s-quarter sq: 4 V, 4 QT, 4 KT."""
        s0 = sq * 512

        def v_chain(sc2):
            def emit():
                sc = 4 * sq + sc2
                ps = projps.tile([128, 512], F32, tag="pp", name=f"psv{sq}_{sc2}")
                for k in range(8):
                    nc.tensor.matmul(
                        out=ps,
                        lhsT=xts[k][:, sc2 * 128 : (sc2 + 1) * 128],
                        rhs=wv_sb[:, k, :],
                        start=(k == 0),
                        stop=(k == 7),
                    )
                nc.vector.tensor_copy(
                    out=v_sb[:, sc, :, 0:64],
                    in_=ps.rearrange("p (h d) -> p h d", d=64),
                )
            return emit

        def q_chain(m):
            def emit():
                ps = projps.tile([128, 512], F32, tag="pp", name=f"psq{sq}_{m}")
                for k in range(8):
                    nc.tensor.matmul(
                        out=ps,
                        lhsT=wq_sb[:, k, m * 128 : (m + 1) * 128],
                        rhs=xts[k],
                        start=(k == 0),
                        stop=(k == 7),
                    )
                qtt = qtpool.tile([128, 512], FR, tag="qt", name=f"qt{sq}_{m}")
                nc.vector.tensor_copy(out=qtt, in_=ps)
                qts[(sq, m)] = qtt
            return emit

        def k_chain(m):
            def emit():
                ps = projps.tile([128, 512], F32, tag="pp", name=f"psk{sq}_{m}")
                for k in range(8):
                    nc.tensor.matmul(
                        out=ps,
                        lhsT=wk_sb[:, k, m * 128 : (m + 1) * 128],
                        rhs=xts[k],
                        start=(k == 0),
                        stop=(k == 7),
                    )
                nc.vector.tensor_copy(out=kt_sb[:, m, s0 : s0 + 512], in_=ps)
            return emit

        # Q first so wave sq-1's tail can overlap; K/V next
        return (
            [q_chain(m) for m in range(NP)]
            + [k_chain(m) for m in range(NP)]
            + [v_chain(c) for c in range(4)]
        )

    wo_sb = None
    stg_pool = None

    def oproj_chain(sc, n):
        def emit():
            ps = projps.tile([128, 512], F32, tag="pp", name=f"pso{sc}_{n}")
            for kp in range(4):
                nc.tensor.matmul(
                    out=ps,
                    lhsT=ctx_sb[:, kp, sc * 128 : (sc + 1) * 128],
                    rhs=wo_sb[:, kp, n * 512 : (n + 1) * 512],
                    start=(kp == 0),
                    stop=(kp == 3),
                )
            st = stg_pool.tile([128, 512], FR, tag="stg", name=f"st{sc}_{n}")
            nc.vector.tensor_copy(out=st, in_=ps)
            nc.sync.dma_start(
                out=pout[sc * 128 : (sc + 1) * 128, n * 512 : (n + 1) * 512],
                in_=st,
            )
        return emit

    def attention_wave(t, fillers):
        """Emit wave t's attention groups, weaving `fillers` chain-emitters
        between k-groups."""
        q0 = t * 512
        ngroups = 2 * (t + 1)  # k-groups of 2 k-tiles
        total_groups = NP * ngroups
        gi = 0
        nf = len(fillers)
        fi = 0
        def _emit_av(exp_t, g, p, cps):
            for hh in range(2):
                for kk in range(2):
                    j = 2 * g + kk
                    nc.tensor.matmul(
                        out=cps[hh],
                        lhsT=v_sb[:, j, 2 * p + hh, :],
                        rhs=exp_t[hh][:, kk * 512 : (kk + 1) * 512],
                        start=(g == 0 and kk == 0),
                        stop=(g == ngroups - 1 and kk == 1),
                    )

        def _normalize(p, cps):
            # stage the raw ctx to SBUF immediately so the PSUM accumulator
            # bank frees before the denominator's DRAM round-trip completes
            for hh in range(2):
                h64 = hh * 64
                rc = recip_pool.tile([1, 512], F32, tag="recip", name=f"rc{p}{t}{hh}", bufs=1)
                nc.vector.reciprocal(out=rc, in_=cps[hh][64:65, :])
                cstg = recip_pool.tile(
                    [64, 512], F32, tag="cstg", name=f"cs{p}{t}{hh}"
                )
                nc.vector.tensor_copy(out=cstg, in_=cps[hh][0:64, :])
                u = (p * 4 + t) * 2 + hh
                nc.sync.dma_start(out=zscratch[u : u + 1, :], in_=rc)
                rcb = recip_pool.tile(
                    [64, 512], F32, tag="recipb", name=f"rcb{p}{t}{hh}"
                )
                nc.sync.dma_start(
                    out=rcb, in_=zscratch[u : u + 1, :].partition_broadcast(64)
                )
                nc.vector.tensor_mul(
                    out=ctx_sb[h64 : h64 + 64, p, q0 : q0 + 512],
                    in0=cstg,
                    in1=rcb,
                )

        pending = None  # (exp_t, g, p, ctx_ps)
        ctx_ps = None
        for p in range(NP):
            ctx_ps = [
                ctxps.tile([65, 512], F32, tag="ctxps", name=f"ctxps{p}_{t}_{i}")
                for i in range(2)
            ]
            for g in range(ngroups):
                # weave fillers evenly across the wave
                while fi < nf and fi * total_groups <= gi * nf:
                    fillers[fi]()
                    fi += 1
                gi += 1
                sc_ps = [
                    scoresps.tile(
                        [128, 1024], F32, tag="scores", name=f"sc{p}_{t}_{g}_{i}"
                    )
                    for i in range(2)
                ]
                for kk in range(2):
                    j = 2 * g + kk
                    for hh in range(2):
                        h64 = hh * 64
                        nc.tensor.matmul(
                            out=sc_ps[hh][:, kk * 512 : (kk + 1) * 512],
                            lhsT=kt_sb[h64 : h64 + 64, p, j * 128 : (j + 1) * 128],
                            rhs=qts[(t, p)][h64 : h64 + 64, :],
                            start=True,
                            stop=True,
                        )
                exp_t = [None, None]
                for hh in range(2):
                    et = expt_pool.tile(
                        [128, 1024], FR, tag="expt", name=f"et{p}_{t}_{g}_{hh}"
                    )
                    nc.scalar.activation(
                        out=et, in_=sc_ps[hh], func=AF.Exp, scale=0.125
                    )
                    exp_t[hh] = et
                if g >= 2 * t:  # diagonal band -> zero causal upper triangle
                    # valid iff qf - kp - 128*(2*(g-2t) + kk) >= 0
                    for hh in range(2):
                        nc.gpsimd.affine_select(
                            out=exp_t[hh],
                            in_=exp_t[hh],
                            compare_op=mybir.AluOpType.is_ge,
                            fill=0.0,
                            base=-256 * (g - 2 * t),
                            pattern=[[-128, 2], [1, 512]],
                            channel_multiplier=-1,
                        )
                # software pipeline: issue the PREVIOUS group's AV matmuls so
                # the PE never sits on this group's exp latency; when that
                # was a pair's last group, its normalization follows
                if pending is not None:
                    _emit_av(*pending)
                    if pending[1] == ngroups - 1:
                        _normalize(pending[2], pending[3])
                pending = (exp_t, g, p, ctx_ps)
        if pending is not None:
            _emit_av(*pending)
            _normalize(pending[2], pending[3])
            pending = None
        # leftover fillers
        while fi < nf:
            fillers[fi]()
            fi += 1

    # quarter 0 projections run un-woven (nothing to overlap with yet)
    xts0 = [
        xtpool.tile([128, 512], FR, tag="xt", name=f"xt0_{k}") for k in range(8)
    ]
    _load_wq_and_xt0(xts0)
    xts1 = load_xt_quarter(1)  # queued before wk/wv: needed by wave 0's fillers
    _load_wkv()
    for emit in proj_chains(0, xts0):
        emit()
    # waves 0..2 weave the next quarter's projection chains
    xts_next = xts1
    for t in range(3):
        chains = proj_chains(t + 1, xts_next)
        if t + 2 <= 3:
            pass
        attention_wave(t, chains)
        if t + 2 <= 3:
            xts_next = load_xt_quarter(t + 2)
    # weights for q/k/v no longer needed; free for the output projection
    wstack.close()
    ostack = stack.enter_context(ExitStack())
    opool = ostack.enter_context(tc.tile_pool(name="opool", bufs=1))
    stg_pool = ostack.enter_context(tc.tile_pool(name="stg", bufs=3))
    wo_sb = opool.tile([128, 4, E], FR, tag="wo")
    nc.sync.dma_start(out=wo_sb, in_=wo.rearrange("(k p) n -> p k n", p=128))
    # wave 3 weaves output-projection chains for s-chunks 0..11 (q < 1536,
    # whose ctxT rows are complete after waves 0..2)
    fillers3 = [oproj_chain(sc, n) for sc in range(12) for n in range(2)]
    # hold back twelve independent chains to cover the final normalize latency
    held = fillers3[-12:]
    attention_wave(3, fillers3[:-12])
    for emit in held:
        emit()
    # tail: s-chunks 12..15 need wave 3's ctxT
    for sc in range(12, 16):
        for n in range(2):
            oproj_chain(sc, n)()
    # sum the two head-group partials across each pair on device; core 2b
    # keeps rows [0, S/2), core 2b+1 rows [S/2, S)
    nc.gpsimd.collective_compute(
        "ReduceScatter", mybir.AluOpType.add,
        replica_groups=[[0, 1], [2, 3], [4, 5], [6, 7]],
        ins=[pout], outs=[rsout],
    )
    nc.sync.dma_start(out=out, in_=rsout)


def _build():
    global _NC
    if _NC is None:
        nc = bacc.Bacc(
            "TRN2", target_bir_lowering=False, debug=False, num_devices=8
        )
        with tile.TileContext(nc) as tc, ExitStack() as stack:
            _emit(tc, stack)
        if not nc.is_finalized():
            nc.finalize()
        _NC = nc
    return _NC


def kernel(X, Wq, Wk, Wv, Wo, bo):
    global _LAST_RESULTS
    X = np.ascontiguousarray(np.asarray(X, dtype=np.float32))
    Wq = np.asarray(Wq, dtype=np.float32)
    Wk = np.asarray(Wk, dtype=np.float32)
    Wv = np.asarray(Wv, dtype=np.float32)
    Wo = np.asarray(Wo, dtype=np.float32)
    bo = np.asarray(bo, dtype=np.float32)

    nc = _build()
    XT = X.transpose(0, 2, 1).astype(np.float16)  # [B, E, S], contiguous
    Wq16 = Wq.astype(np.float16)
    Wk16 = Wk.astype(np.float16)
    Wv16 = Wv.astype(np.float16)
    Wo16 = Wo.astype(np.float16)
    # wpack[g]: [4E, HDC] = [Wq[:,cs]; Wk[:,cs]; Wv[:,cs]; Wo[cs,:] as [E,HDC]]
    wpacks = []
    for g in range(2):
        cs = slice(g * HDC, (g + 1) * HDC)
        wpacks.append(
            np.concatenate(
                [
                    Wq16[:, cs],
                    Wk16[:, cs],
                    Wv16[:, cs],
                    Wo16[cs, :].reshape(E, HDC),
                ],
                axis=0,
            )
        )
    vones = np.ones((16, NHC), dtype=np.float16)
    in_maps = []
    for c in range(8):
        b, g = c // 2, c % 2
        in_maps.append(
            {
                # pair rank (c%2) contributes E-rows [rank*512, (rank+1)*512)
                "xsh": XT[b, (c % 2) * (E // 2) : (c % 2 + 1) * (E // 2)],
                # quad rank (c//2) contributes pack rows [rank*E, (rank+1)*E)
                "wsh": wpacks[g][(c // 2) * E : (c // 2 + 1) * E],
                "vones": vones,
            }
        )
    trace = bool(int(os.environ.get("KTRACE", "0")))
    res = run_bass_kernel_spmd(
        nc, in_maps, core_ids=list(range(8)), trace=trace
    )
    _LAST_RESULTS = res
    out = np.empty((B, S, E), dtype=np.float32)
    for b in range(B):
        out[b, : S // 2] = res.results[2 * b]["out"]
        out[b, S // 2 :] = res.results[2 * b + 1]["out"]
    out += bo
    return out

